# revision 1
# baseline (speedup 1.0000x reference)
"""DeepFM forward on 8 Trainium2 NeuronCores (Bass/Tile).

Strategy: data-parallel over the batch (2048 samples/core), embedding +
fm1 tables concatenated into one [F*V, 17] fp32 table replicated to every
core and gathered on-device with indirect DMAs (128 rows/instruction).
The DNN runs in transposed layout (features/hidden on partitions, samples
on the free dim); BatchNorm batch statistics are produced with fused
ACT copy/square+accumulate ops and globalized with two tiny AllReduces.

Self-contained: hardcodes all shapes from the problem spec.
"""

import numpy as np

import concourse.bass as bass
import concourse.mybir as mybir
import concourse.tile as tile
from concourse.bass_utils import run_bass_kernel_spmd
from concourse.masks import make_identity

B, F, V, K, D, H = 16384, 26, 100000, 16, 13, 400
E = K + 1            # 17 floats per table row (16 emb + 1 fm1)
NCORES = 8
BL = B // NCORES     # 2048 samples per core
NT = BL // 128       # 16 sample tiles of 128
NSG = BL // 512      # 4 sample groups of 512
EPS = 1e-5
FP = mybir.dt.float32

# W1 contraction chunks over 429 = 3*128 + 45; hidden chunks over 400 = 3*128 + 16
KC1 = [128, 128, 128, 45]
KC2 = [128, 128, 128, 16]
HC = [128, 128, 128, 16]

_cached = {}


def _split_multi_waits(nc, max_waits=1):
    """This walrus build rejects instructions carrying >1 semaphore wait.
    Re-emit extra waits as standalone single-wait sem nops on the same
    engine immediately before the instruction (same per-engine order, so
    semantics are unchanged)."""
    for bb in nc.main_func.blocks:
        insts = bb.instructions
        new_list = []
        changed = False
        for inst in insts:
            si = inst.sync_info
            waits = list(si.on_wait) if si is not None and si.on_wait else []
            sem_waits = [w for w in waits if w.wait_reg is None]
            reg_waits = [w for w in waits if w.wait_reg is not None]
            if len(waits) > max_waits and sem_waits:
                keep = max(0, max_waits - len(reg_waits))
                move = sem_waits[: len(sem_waits) - keep]
                kept = sem_waits[len(sem_waits) - keep:]
                for w in move:
                    nop = mybir.InstEventSemaphore(
                        name=nc.get_next_instruction_name(),
                        engine=inst.engine,
                        ins=[], outs=[],
                        sync_info=mybir.SyncInfo(on_wait=[w], on_update=[]),
                    )
                    nc.register_instruction(nop, overwrite=True)
                    new_list.append(nop)
                    changed = True
                si.on_wait = reg_waits + kept
            new_list.append(inst)
        if changed:
            insts.clear()
            insts.extend(new_list)


def _build():
    nc = bass.Bass("TRN2", target_bir_lowering=False, debug=False,
                   num_devices=NCORES, dynamic_dma_scratch_size=65536)
    A = mybir.AluOpType
    AF = mybir.ActivationFunctionType

    tbl = nc.dram_tensor("tbl", [F * V, E], FP, kind="ExternalInput").ap()
    idxs = nc.dram_tensor("idxs", [128, NT * F], mybir.dt.int32,
                          kind="ExternalInput").ap()
    xdt = nc.dram_tensor("xdt", [D, BL], mybir.dt.float32r, kind="ExternalInput").ap()
    xd3 = nc.dram_tensor("xd3", [128, NT * D], FP, kind="ExternalInput").ap()
    w1 = nc.dram_tensor("w1", [429, H], mybir.dt.float32r, kind="ExternalInput").ap()
    w2 = nc.dram_tensor("w2", [H, H], mybir.dt.float32r, kind="ExternalInput").ap()
    w3 = nc.dram_tensor("w3", [H, 1], mybir.dt.float32r, kind="ExternalInput").ap()
    wd = nc.dram_tensor("wd", [128, D], FP, kind="ExternalInput").ap()
    gb1 = nc.dram_tensor("gb1", [128, 8], FP, kind="ExternalInput").ap()
    gb2 = nc.dram_tensor("gb2", [128, 8], FP, kind="ExternalInput").ap()
    bias = nc.dram_tensor("bias", [128, 1], FP, kind="ExternalInput").ap()
    out_d = nc.dram_tensor("out", [128, NT], FP, kind="ExternalOutput").ap()

    with tile.TileContext(nc) as tc:
        with (
            tc.tile_pool(name="const", bufs=1) as cpool,
            tc.tile_pool(name="xt", bufs=1) as xtpool,
            tc.tile_pool(name="zb", bufs=1) as zpool,
            tc.tile_pool(name="hb", bufs=1) as hpool,
            tc.tile_pool(name="g", bufs=8) as gpool,
            tc.tile_pool(name="sm", bufs=3) as smpool,
            tc.tile_pool(name="zq", bufs=2) as zqpool,
            tc.tile_pool(name="ps", bufs=4, space="PSUM") as pspool,
            tc.tile_pool(name="pst", bufs=4, space="PSUM") as pstpool,
            tc.tile_pool(name="dram", bufs=1, space="DRAM") as dpool,
        ):
            # ---- constants / weights ----
            idxs_t = cpool.tile([128, NT * F], mybir.dt.int32)
            nc.sync.dma_start(out=idxs_t[:], in_=idxs[:, :])
            ident = cpool.tile([128, 128], FP)
            make_identity(nc, ident[:])
            w1_sb = []
            off = 0
            for kc in range(4):
                w1_c = cpool.tile([KC1[kc], H], mybir.dt.float32r, tag=f"w1_{kc}")
                nc.sync.dma_start(out=w1_c[:], in_=w1[off:off + KC1[kc], :])
                w1_sb.append(w1_c)
                off += KC1[kc]
            w2_sb = []
            off = 0
            for kc in range(4):
                w2_c = cpool.tile([KC2[kc], H], mybir.dt.float32r, tag=f"w2_{kc}")
                nc.sync.dma_start(out=w2_c[:], in_=w2[off:off + KC2[kc], :])
                w2_sb.append(w2_c)
                off += KC2[kc]
            w3_sb = []
            off = 0
            for kc in range(4):
                w3_c = cpool.tile([KC2[kc], 1], mybir.dt.float32r, tag=f"w3_{kc}")
                nc.sync.dma_start(out=w3_c[:], in_=w3[off:off + KC2[kc], :])
                w3_sb.append(w3_c)
                off += KC2[kc]
            wd_sb = cpool.tile([128, D], FP)
            nc.sync.dma_start(out=wd_sb[:], in_=wd[:, :])
            gb1_sb = cpool.tile([128, 8], FP)
            nc.sync.dma_start(out=gb1_sb[:], in_=gb1[:, :])
            gb2_sb = cpool.tile([128, 8], FP)
            nc.sync.dma_start(out=gb2_sb[:], in_=gb2[:, :])
            bias_sb = cpool.tile([128, 1], FP)
            nc.sync.dma_start(out=bias_sb[:], in_=bias[:, :])
            xd3_sb = cpool.tile([128, NT, D], FP)
            nc.sync.dma_start(out=xd3_sb[:], in_=xd3[:, :].rearrange(
                "p (t d) -> p t d", d=D))
            one_sb = cpool.tile([1, 1], FP)
            nc.vector.memset(one_sb[:], 1.0)
            eps_sb = cpool.tile([128, 1], FP)
            nc.vector.memset(eps_sb[:], EPS)

            # ---- xT chunks (feature-major activations) ----
            xt_sb = []
            for _i in range(3):
                xt_c = xtpool.tile([128, BL], mybir.dt.float32r, tag=f"xt{_i}")
                xt_sb.append(xt_c)
            xt_c3 = xtpool.tile([48, BL], mybir.dt.float32r, tag="xt3")
            xt_sb.append(xt_c3)
            # dense features live on partitions 32..44 of chunk 3
            nc.sync.dma_start(out=xt_sb[3][32:45, :], in_=xdt[:, :])

            # per-sample scalar accumulators, one column per sample tile
            y1st = cpool.tile([128, NT], FP)
            fmy = cpool.tile([128, NT], FP)
            snorm = cpool.tile([128, NT], FP)
            esq = cpool.tile([128, NT], FP)

            # ---- gather + first-order + transposes, per 128-sample tile ----
            for t in range(NT):
                g = gpool.tile([128, F, E], FP, tag="g")
                for f in range(F):
                    nc.gpsimd.indirect_dma_start(
                        out=g[:][:, f:f + 1, :].rearrange("p a b -> p (a b)"),
                        out_offset=None, in_=tbl[:],
                        in_offset=bass.IndirectOffsetOnAxis(
                            ap=idxs_t[:, t * F + f: t * F + f + 1], axis=0),
                    )
                emb_pfk = g[:][:, :, 0:K]                              # [128,26,16]
                # contiguous copy of the emb block (PE transpose needs a
                # single-free-dim rhs AP; also speeds the reads below)
                embc = smpool.tile([128, F * K], FP, tag="embc")
                nc.vector.tensor_copy(
                    out=embc[:].rearrange("p (f k) -> p f k", k=K),
                    in_=emb_pfk)
                # s_k = sum_f emb[f,k]
                s_t = smpool.tile([128, K], FP, tag="s")
                nc.vector.tensor_reduce(
                    out=s_t[:],
                    in_=embc[:].rearrange("p (f k) -> p k f", k=K),
                    axis=mybir.AxisListType.X, op=A.add)
                # sum_f,k emb^2 (ACT square with accumulate)
                sq_scr = smpool.tile([128, F * K], FP, tag="sqscr")
                nc.scalar.activation(out=sq_scr[:], in_=embc[:],
                                     func=AF.Square,
                                     accum_out=esq[:, t:t + 1])
                # |s|^2
                s_scr = smpool.tile([128, K], FP, tag="sscr")
                nc.scalar.activation(out=s_scr[:], in_=s_t[:], func=AF.Square,
                                     accum_out=snorm[:, t:t + 1])
                # first-order: sum_f fm1 + X_dense @ Wd
                fm1_t = smpool.tile([128, 1], FP, tag="fm1")
                nc.vector.tensor_reduce(
                    out=fm1_t[:],
                    in_=g[:][:, :, K:K + 1].rearrange("p f k -> p (k f)"),
                    axis=mybir.AxisListType.X, op=A.add)
                dd_scr = smpool.tile([128, D], FP, tag="ddscr")
                nc.vector.scalar_tensor_tensor(
                    out=dd_scr[:], in0=xd3_sb[:, t, :], scalar=1.0,
                    in1=wd_sb[:], op0=A.mult,
                    op1=A.mult, accum_out=y1st[:, t:t + 1])
                nc.vector.tensor_tensor(out=y1st[:, t:t + 1],
                                        in0=y1st[:, t:t + 1], in1=fm1_t[:],
                                        op=A.add)
                # transposes into xT chunks
                for kc in range(3):
                    pst = pstpool.tile([128, 128], FP, tag="pst", space="PSUM")
                    nc.tensor.transpose(
                        out=pst[:], in_=embc[:, kc * 128:(kc + 1) * 128],
                        identity=ident[:])
                    nc.vector.tensor_copy(
                        out=xt_sb[kc][:, t * 128:(t + 1) * 128], in_=pst[:])
                pst = pstpool.tile([128, 128], FP, tag="pst", space="PSUM")
                nc.tensor.transpose(out=pst[0:32, :],
                                    in_=embc[:, 384:416], identity=ident[:])
                nc.vector.tensor_copy(out=xt_sb[3][0:32, t * 128:(t + 1) * 128],
                                      in_=pst[0:32, :])

            # fm_y = 0.5*(|s|^2 - sum emb^2)
            fm_tmp = cpool.tile([128, NT], FP)
            nc.vector.tensor_tensor(out=fm_tmp[:], in0=snorm[:], in1=esq[:],
                                    op=A.subtract)
            nc.vector.tensor_scalar_mul(fmy[:], fm_tmp[:], 0.5)
            # comb = y1st + fmy + (bd+b3)
            comb = cpool.tile([128, NT], FP)
            nc.vector.scalar_tensor_tensor(out=comb[:], in0=y1st[:],
                                           scalar=bias_sb[:, 0:1], in1=fmy[:],
                                           op0=A.add, op1=A.add)

            def dnn_layer(w_sb, kcs, x_chunks, gb_sb, ztag, htag, zpool_, hpool_, h_dt=FP):
                """z = x @ W (transposed layout); returns h tiles after BN+relu."""
                z_sb = []
                for hc in range(4):
                    z_c = zpool_.tile([HC[hc], BL], FP, tag=f"zc{hc}")
                    z_sb.append(z_c)
                sums = cpool.tile([128, 16], FP, tag=f"{ztag}sum")
                sqs = cpool.tile([128, 16], FP, tag=f"{ztag}sq")
                nc.vector.memset(sums[:], 0.0)
                nc.vector.memset(sqs[:], 0.0)
                for sg in range(NSG):
                    for hc in range(4):
                        hdim = HC[hc]
                        psz = pspool.tile([128, 512], FP, tag="psz",
                                          space="PSUM")
                        for kc in range(4):
                            nc.tensor.matmul(
                                out=psz[:hdim, :],
                                lhsT=w_sb[kc][:, hc * 128:hc * 128 + hdim],
                                rhs=x_chunks[kc][:kcs[kc],
                                                 sg * 512:(sg + 1) * 512],
                                start=(kc == 0), stop=(kc == 3))
                        col = hc * 4 + sg
                        nc.scalar.activation(
                            out=z_sb[hc][:, sg * 512:(sg + 1) * 512],
                            in_=psz[:hdim, :], func=AF.Copy,
                            accum_out=sums[:hdim, col:col + 1])
                        zq = zqpool.tile([128, 512], FP, tag="zq")
                        nc.scalar.activation(
                            out=zq[:hdim, :], in_=psz[:hdim, :],
                            func=AF.Square,
                            accum_out=sqs[:hdim, col:col + 1])
                # pack per-hc sums and allreduce
                stats = cpool.tile([128, 8], FP, tag=f"{ztag}st")
                nc.vector.tensor_reduce(
                    out=stats[:, 0:4],
                    in_=sums[:].rearrange("p (h s) -> p h s", s=4),
                    axis=mybir.AxisListType.X, op=A.add)
                nc.vector.tensor_reduce(
                    out=stats[:, 4:8],
                    in_=sqs[:].rearrange("p (h s) -> p h s", s=4),
                    axis=mybir.AxisListType.X, op=A.add)
                in_b = dpool.tile([128, 8], FP, tag=f"{ztag}arin")
                out_b = dpool.tile([128, 8], FP, tag=f"{ztag}arout")
                # bounce DMAs on HWDGE (sync) so the Pool dynamic queue
                # stays exclusively a gather pipe
                nc.sync.dma_start(out=in_b[:], in_=stats[:])
                nc.gpsimd.collective_compute(
                    "AllReduce", A.add,
                    replica_groups=[list(range(NCORES))],
                    ins=[in_b.opt()], outs=[out_b.opt()])
                statg = cpool.tile([128, 8], FP, tag=f"{ztag}sg")
                nc.sync.dma_start(out=statg[:], in_=out_b[:])
                mean = cpool.tile([128, 4], FP, tag=f"{ztag}mean")
                nc.vector.tensor_scalar_mul(mean[:], statg[:, 0:4], 1.0 / B)
                msq = cpool.tile([128, 4], FP, tag=f"{ztag}msq")
                nc.vector.tensor_tensor(out=msq[:], in0=mean[:], in1=mean[:],
                                        op=A.mult)
                var = cpool.tile([128, 4], FP, tag=f"{ztag}var")
                nc.vector.scalar_tensor_tensor(
                    out=var[:], in0=statg[:, 4:8], scalar=1.0 / B, in1=msq[:],
                    op0=A.mult, op1=A.subtract)
                std = cpool.tile([128, 4], FP, tag=f"{ztag}std")
                nc.scalar.activation(out=std[:], in_=var[:], func=AF.Sqrt,
                                     bias=eps_sb[:])
                rstd = cpool.tile([128, 4], FP, tag=f"{ztag}rstd")
                nc.vector.reciprocal(out=rstd[:], in_=std[:])
                a_sc = cpool.tile([128, 4], FP, tag=f"{ztag}a")
                nc.vector.tensor_tensor(out=a_sc[:], in0=gb_sb[:, 0:4],
                                        in1=rstd[:], op=A.mult)
                am = cpool.tile([128, 4], FP, tag=f"{ztag}am")
                nc.vector.tensor_tensor(out=am[:], in0=a_sc[:], in1=mean[:],
                                        op=A.mult)
                shift = cpool.tile([128, 4], FP, tag=f"{ztag}sh")
                nc.vector.tensor_tensor(out=shift[:], in0=gb_sb[:, 4:8],
                                        in1=am[:], op=A.subtract)
                h_sb = []
                for hc in range(4):
                    h_c = hpool_.tile([HC[hc], BL], h_dt, tag=f"{htag}{hc}")
                    h_sb.append(h_c)
                for hc in range(4):
                    hdim = HC[hc]
                    for sg in range(NSG):
                        nc.scalar.activation(
                            out=h_sb[hc][:, sg * 512:(sg + 1) * 512],
                            in_=z_sb[hc][:, sg * 512:(sg + 1) * 512],
                            func=AF.Relu, scale=a_sc[:hdim, hc:hc + 1],
                            bias=shift[:hdim, hc:hc + 1])
                return h_sb

            h1_sb = dnn_layer(w1_sb, KC1, xt_sb, gb1_sb, "z1", "xt",
                              zpool, xtpool, h_dt=mybir.dt.float32r)
            h2_sb = dnn_layer(w2_sb, KC2, h1_sb, gb2_sb, "z2", "h2c",
                              zpool, hpool, h_dt=mybir.dt.float32r)

            # ---- output head: dnn_y^T then transpose back and combine ----
            out_sb = cpool.tile([128, NT], FP)
            for sg in range(NSG):
                psy = pstpool.tile([1, 512], FP, tag="pst", space="PSUM")
                for kc in range(4):
                    nc.tensor.matmul(
                        out=psy[:, :],
                        lhsT=w3_sb[kc][:],
                        rhs=h2_sb[kc][:KC2[kc], sg * 512:(sg + 1) * 512],
                        start=(kc == 0), stop=(kc == 3))
                ypre = smpool.tile([1, 512], FP, tag="ypre")
                nc.vector.tensor_copy(out=ypre[:], in_=psy[:])
                for sub in range(4):
                    t = sg * 4 + sub
                    psf = pstpool.tile([128, 1], FP, tag="pst", space="PSUM")
                    nc.tensor.matmul(
                        out=psf[:], lhsT=ypre[0:1, sub * 128:(sub + 1) * 128],
                        rhs=one_sb[:], start=True, stop=True)
                    nc.vector.tensor_tensor(out=out_sb[:, t:t + 1],
                                            in0=psf[:], in1=comb[:, t:t + 1],
                                            op=A.add)
            nc.sync.dma_start(out=out_d[:, :], in_=out_sb[:])

    _split_multi_waits(nc)
    return nc


def _prep_core(c, X_cat, X_dense, Wd_, bd_, g1_, be1_, g2_, be2_, b3_):
    sl = slice(c * BL, (c + 1) * BL)
    xc = np.asarray(X_cat[sl], dtype=np.int64)
    # idxs[p, t*F+f] = f*V + X_cat[c*BL + t*128 + p, f]
    gidx = (xc + np.arange(F, dtype=np.int64)[None, :] * V).astype(np.int32)
    idxs = np.ascontiguousarray(
        gidx.reshape(NT, 128, F).transpose(1, 0, 2).reshape(128, NT * F))
    xd = np.asarray(X_dense[sl], dtype=np.float32)
    xdt = np.ascontiguousarray(xd.T)
    xd3 = np.ascontiguousarray(
        xd.reshape(NT, 128, D).transpose(1, 0, 2).reshape(128, NT * D))
    return {"idxs": idxs, "xdt": xdt, "xd3": xd3}


def _pack_gb(g, b):
    out = np.zeros((128, 8), np.float32)
    gp = np.zeros(512, np.float32)
    bp = np.zeros(512, np.float32)
    gp[:H] = g
    bp[:H] = b
    out[:, 0:4] = gp.reshape(4, 128).T
    out[:, 4:8] = bp.reshape(4, 128).T
    return out


def kernel(X_cat, X_dense, fm1_tables, emb_tables, Wd, bd,
           W1, b1, g1, be1, W2, b2, g2, be2, W3, b3):
    if "nc" not in _cached:
        _cached["nc"] = _build()
    nc = _cached["nc"]

    tbl = np.concatenate(
        [np.asarray(emb_tables, np.float32).reshape(F * V, K),
         np.asarray(fm1_tables, np.float32).reshape(F * V, 1)],
        axis=1)
    tbl = np.ascontiguousarray(tbl)
    shared = {
        "tbl": tbl,
        "w1": np.ascontiguousarray(np.asarray(W1, np.float32)),
        "w2": np.ascontiguousarray(np.asarray(W2, np.float32)),
        "w3": np.ascontiguousarray(np.asarray(W3, np.float32)),
        "wd": np.ascontiguousarray(np.broadcast_to(np.asarray(Wd, np.float32).reshape(1, D), (128, D))),
        "gb1": _pack_gb(np.asarray(g1, np.float32), np.asarray(be1, np.float32)),
        "gb2": _pack_gb(np.asarray(g2, np.float32), np.asarray(be2, np.float32)),
        "bias": np.full((128, 1), float(np.asarray(bd).reshape(-1)[0])
                        + float(np.asarray(b3).reshape(-1)[0]), np.float32),
    }
    in_maps = []
    for c in range(NCORES):
        m = dict(shared)
        m.update(_prep_core(c, X_cat, X_dense, Wd, bd, g1, be1, g2, be2, b3))
        in_maps.append(m)

    res = run_bass_kernel_spmd(nc, in_maps, core_ids=list(range(NCORES)))
    y = np.empty((B, 1), np.float32)
    for c in range(NCORES):
        o = res.results[c]["out"]            # [128, NT]
        y[c * BL:(c + 1) * BL, 0] = o.T.reshape(BL)
    return y

